# revision 1
# baseline (speedup 1.0000x reference)
"""Linformer attention TRN2 Bass kernel.

Sharding: 8 cores = 4 batches x 2 head-groups (8 heads / 512 cols each).
Per-core math (all matmuls fp16 inputs, fp32 PSUM accumulation):
  G  = x^T E, H = x^T F            (l-contraction, x natural layout)
  kE = Wk^T G + bk (x) sE          ([dg, m], no k materialization)
  vF = H^T Wv + sF (x) bv          ([m, dg], no v materialization)
  qT = Wq^T x^T + bq               ([n, l], x^T via DMA-transpose)
  qk_h = qT_h^T kE_h               ([l, m] per head, K=dh=64)
  attn = softmax(qk) (ACT exp with fused row-sum), normalized, PE-transposed
  outT_h = vF_h^T attn^T           ([dh, l])
  y = outT^T Wo                    ([l, D] partial; host sums the 2 groups + bo)
The logit chain (G, kE, qT, qk) uses fp16 hi/lo splitting (3 matmuls each)
for ~fp32 accuracy; the value chain (H, vF, out, y) is plain fp16.
"""

import numpy as np

B, L, D, H = 4, 4096, 1024, 16
DH = D // H          # 64
KP = 256             # Linformer projection dim
NG = 512             # per-core head-group width (8 heads * 64)
LC = 512             # l-chunk
NCHUNK = L // LC     # 8
LT = L // 128        # 32 l-tiles
DT = D // 128        # 8 d-tiles
SCALE = DH ** -0.5

_CACHE = {}


def _split16(a):
    hi = np.asarray(a, np.float32).astype(np.float16)
    lo = (np.asarray(a, np.float32) - hi.astype(np.float32)).astype(np.float16)
    return hi, lo


def _build():
    import concourse.bass as bass
    from concourse import bacc
    import concourse.mybir as mybir
    import concourse.tile as tile
    from concourse.masks import make_identity

    f16 = mybir.dt.float16
    f32 = mybir.dt.float32
    AF = mybir.ActivationFunctionType
    AX = mybir.AxisListType

    nc = bacc.Bacc(trn_type="TRN2", target_bir_lowering=False, debug=False,
                   enable_asserts=False)

    def din(name, shape):
        return nc.dram_tensor(name, shape, f16, kind="ExternalInput").ap()

    xhi_d = din("xhi", [L, D])
    xlo_d = din("xlo", [L, D])
    efhi_d = din("efhi", [L, 2 * KP])
    elo_d = din("elo", [L, KP])
    wqh_d = din("wqh", [D, NG])
    wql_d = din("wql", [D, NG])
    wkh_d = din("wkh", [D, NG])
    wkl_d = din("wkl", [D, NG])
    wv_d = din("wv", [D, NG])
    wo_d = din("wo", [NG, D])
    bqs_d = din("bqs", [1, NG])
    bk_d = din("bk", [1, NG])
    bv_d = din("bv", [1, NG])
    se_d = din("se", [1, KP])
    sf_d = din("sf", [1, KP])
    y_d = nc.dram_tensor("y", [L, D], f16, kind="ExternalOutput").ap()

    with tile.TileContext(nc) as tc:
        with (
            tc.tile_pool(name="const", bufs=1) as cp,
            tc.tile_pool(name="wts", bufs=1) as wp,
            tc.tile_pool(name="ghsb", bufs=1) as gp,
            tc.tile_pool(name="kvsb", bufs=1) as kp,
        ):
            ident = cp.tile([128, 128], f16, name="ident", tag="ident")
            make_identity(nc, ident[:])
            ones = cp.tile([1, LC], f16, name="ones", tag="ones")
            nc.vector.memset(ones[:], 1.0)
            vecs = {}
            for nm, dr, w in (("bqs", bqs_d, NG), ("bk", bk_d, NG),
                              ("bv", bv_d, NG), ("se", se_d, KP), ("sf", sf_d, KP)):
                t = cp.tile([1, w], f16, tag=nm)
                nc.gpsimd.dma_start(t[:], dr[0:1, :])
                vecs[nm] = t

            def load_w(name, dr, cols):
                ts = []
                for dt in range(dr.shape[0] // 128):
                    t = wp.tile([128, cols], f16, name=f"{name}{dt}", tag=f"{name}{dt}")
                    nc.gpsimd.dma_start(t[:], dr[dt * 128:(dt + 1) * 128, :])
                    ts.append(t)
                return ts

            wqh = load_w("wqh", wqh_d, NG)
            wql = load_w("wql", wql_d, NG)
            wkh = load_w("wkh", wkh_d, NG)
            wkl = load_w("wkl", wkl_d, NG)
            wv = load_w("wv", wv_d, NG)
            wo = load_w("wo", wo_d, D)

            # ---------------- Phase A: G/H accumulation ----------------
            ghi = [gp.tile([128, KP], f16, name=f"ghi{dt}", tag=f"ghi{dt}") for dt in range(DT)]
            glo = [gp.tile([128, KP], f16, name=f"glo{dt}", tag=f"glo{dt}") for dt in range(DT)]
            h16 = [gp.tile([128, KP], f16, name=f"h{dt}", tag=f"h{dt}") for dt in range(DT)]
            with (
                tc.tile_pool(name="ghps", bufs=1, space="PSUM") as ghp,
                tc.tile_pool(name="xa", bufs=4) as xap,
                tc.tile_pool(name="efa", bufs=4) as efp,
            ):
                GH = [ghp.tile([128, 2 * KP], f32, name=f"gh{dt}", tag=f"gh{dt}") for dt in range(DT)]
                for lt in range(LT):
                    r = slice(lt * 128, (lt + 1) * 128)
                    xh = xap.tile([128, D], f16, name="xh", tag="xh")
                    nc.gpsimd.dma_start(xh[:], xhi_d[r, :])
                    xl = xap.tile([128, D], f16, name="xl", tag="xl")
                    nc.gpsimd.dma_start(xl[:], xlo_d[r, :])
                    ef = efp.tile([128, 2 * KP], f16, name="ef", tag="ef")
                    nc.gpsimd.dma_start(ef[:], efhi_d[r, :])
                    el = efp.tile([128, KP], f16, name="el", tag="el")
                    nc.gpsimd.dma_start(el[:], elo_d[r, :])
                    for dt in range(DT):
                        c = slice(dt * 128, (dt + 1) * 128)
                        nc.tensor.matmul(GH[dt][:], lhsT=xh[:, c], rhs=ef[:],
                                         start=(lt == 0), stop=False)
                        nc.tensor.matmul(GH[dt][:, 0:KP], lhsT=xh[:, c], rhs=el[:],
                                         start=False, stop=False)
                        nc.tensor.matmul(GH[dt][:, 0:KP], lhsT=xl[:, c],
                                         rhs=ef[:, 0:KP], start=False,
                                         stop=(lt == LT - 1))
                for dt in range(DT):
                    nc.vector.tensor_copy(ghi[dt][:], GH[dt][:, 0:KP])
                    nc.vector.tensor_sub(glo[dt][:], GH[dt][:, 0:KP], ghi[dt][:])
                    nc.scalar.copy(h16[dt][:], GH[dt][:, KP:2 * KP])

            # ---------------- kE / vF ----------------
            keh = [kp.tile([128, KP], f16, name=f"keh{i}", tag=f"keh{i}") for i in range(4)]
            kel = [kp.tile([128, KP], f16, name=f"kel{i}", tag=f"kel{i}") for i in range(4)]
            vf = [kp.tile([128, NG], f16, name=f"vf{i}", tag=f"vf{i}") for i in range(2)]
            with tc.tile_pool(name="kvps", bufs=2, space="PSUM") as kvp:
                for dgt in range(4):
                    c = slice(dgt * 128, (dgt + 1) * 128)
                    ps = kvp.tile([128, KP], f32, name="keps", tag="keps")
                    for dt in range(DT):
                        nc.tensor.matmul(ps[:], lhsT=wkh[dt][:, c], rhs=ghi[dt][:],
                                         start=(dt == 0), stop=False)
                        nc.tensor.matmul(ps[:], lhsT=wkh[dt][:, c], rhs=glo[dt][:],
                                         start=False, stop=False)
                        nc.tensor.matmul(ps[:], lhsT=wkl[dt][:, c], rhs=ghi[dt][:],
                                         start=False, stop=False)
                    nc.tensor.matmul(ps[:], lhsT=vecs["bk"][0:1, c],
                                     rhs=vecs["se"][0:1, :], start=False, stop=True)
                    nc.vector.tensor_copy(keh[dgt][:], ps[:])
                    nc.vector.tensor_sub(kel[dgt][:], ps[:], keh[dgt][:])
                for mt in range(2):
                    c = slice(mt * 128, (mt + 1) * 128)
                    ps = kvp.tile([128, NG], f32, name="vfps", tag="vfps")
                    for dt in range(DT):
                        nc.tensor.matmul(ps[:], lhsT=h16[dt][:, c], rhs=wv[dt][:],
                                         start=(dt == 0), stop=False)
                    nc.tensor.matmul(ps[:], lhsT=vecs["sf"][0:1, c],
                                     rhs=vecs["bv"][0:1, :], start=False, stop=True)
                    nc.scalar.copy(vf[mt][:], ps[:])

            # ---------------- Phase B: per l-chunk ----------------
            with (
                tc.tile_pool(name="xt", bufs=20) as xtp,
                tc.tile_pool(name="qt", bufs=10) as qtp,
                tc.tile_pool(name="at", bufs=6) as atp,
                tc.tile_pool(name="st", bufs=10) as stp,
                tc.tile_pool(name="ot", bufs=2) as otp,
                tc.tile_pool(name="yo", bufs=4) as yop,
                tc.tile_pool(name="psA", bufs=2, space="PSUM") as psA,
                tc.tile_pool(name="psB", bufs=2, space="PSUM") as psB,
            ):
                for ci in range(NCHUNK):
                    l0 = ci * LC
                    xth, xtl = [], []
                    for dt in range(DT):
                        c = slice(dt * 128, (dt + 1) * 128)
                        t = xtp.tile([128, LC], f16, name="xth", tag="xth")
                        nc.sync.dma_start(t[:], xhi_d[l0:l0 + LC, c], transpose=True)
                        xth.append(t)
                        t = xtp.tile([128, LC], f16, name="xtl", tag="xtl")
                        nc.sync.dma_start(t[:], xlo_d[l0:l0 + LC, c], transpose=True)
                        xtl.append(t)
                    qth, qtl = [], []
                    for nt in range(4):
                        c = slice(nt * 128, (nt + 1) * 128)
                        ps = psA.tile([128, LC], f32, name="qtps", tag="qtps")
                        for dt in range(DT):
                            nc.tensor.matmul(ps[:], lhsT=wqh[dt][:, c], rhs=xth[dt][:],
                                             start=(dt == 0), stop=False)
                            nc.tensor.matmul(ps[:], lhsT=wqh[dt][:, c], rhs=xtl[dt][:],
                                             start=False, stop=False)
                            nc.tensor.matmul(ps[:], lhsT=wql[dt][:, c], rhs=xth[dt][:],
                                             start=False, stop=False)
                        nc.tensor.matmul(ps[:], lhsT=vecs["bqs"][0:1, c],
                                         rhs=ones[0:1, :], start=False, stop=True)
                        th = qtp.tile([128, LC], f16, name="qth", tag="qth")
                        nc.vector.tensor_copy(th[:], ps[:])
                        tl = qtp.tile([128, LC], f16, name="qtl", tag="qtl")
                        nc.vector.tensor_sub(tl[:], ps[:], th[:])
                        qth.append(th)
                        qtl.append(tl)
                    outT = [otp.tile([128, LC], f16, name=f"ot{i}", tag=f"ot{i}") for i in range(4)]
                    for h in range(8):
                        nt, po = h // 2, 64 * (h % 2)
                        pr = slice(po, po + 64)
                        hc = slice(h * 64, (h + 1) * 64)
                        aT = [atp.tile([128, LC], f16, name="aT", tag="aT") for _ in range(2)]
                        for lt in range(4):
                            fc = slice(lt * 128, (lt + 1) * 128)
                            qk = psB.tile([128, KP], f32, name="qkps", tag="qkps")
                            nc.tensor.matmul(qk[:], lhsT=qth[nt][pr, fc],
                                             rhs=keh[nt][pr, :], start=True, stop=False)
                            nc.tensor.matmul(qk[:], lhsT=qth[nt][pr, fc],
                                             rhs=kel[nt][pr, :], start=False, stop=False)
                            nc.tensor.matmul(qk[:], lhsT=qtl[nt][pr, fc],
                                             rhs=keh[nt][pr, :], start=False, stop=True)
                            nmx = stp.tile([128, 1], f32, name="nmx", tag="nmx")
                            nc.vector.reduce_max(nmx[:], qk[:], axis=AX.X, negate=True)
                            attn = atp.tile([128, KP], f16, name="attn", tag="attn")
                            sm = stp.tile([128, 1], f32, name="sm", tag="sm")
                            nc.scalar.activation(attn[:], qk[:], AF.Exp,
                                                 bias=nmx[:], scale=1.0,
                                                 accum_out=sm[:])
                            rcp = stp.tile([128, 1], f32, name="rcp", tag="rcp")
                            nc.vector.reciprocal(rcp[:], sm[:])
                            nc.vector.tensor_scalar_mul(attn[:], attn[:], rcp[:])
                            for mt in range(2):
                                tp = psB.tile([128, 128], f16, name="tps", tag="tps", bufs=1)
                                nc.tensor.transpose(
                                    tp[:], attn[:, mt * 128:(mt + 1) * 128], ident[:])
                                nc.scalar.copy(aT[mt][:, fc], tp[:])
                        op = psA.tile([64, LC], f32, name="otps", tag="otps")
                        nc.tensor.matmul(op[:], lhsT=vf[0][:, hc], rhs=aT[0][:],
                                         start=True, stop=False)
                        nc.tensor.matmul(op[:], lhsT=vf[1][:, hc], rhs=aT[1][:],
                                         start=False, stop=True)
                        nc.vector.tensor_copy(outT[h // 2][pr, :], op[:])
                    for lt in range(4):
                        fc = slice(lt * 128, (lt + 1) * 128)
                        yt = yop.tile([128, D], f16, name="yt", tag="yt")
                        for hf in range(2):
                            ps = psA.tile([128, LC], f32, name="yps", tag="yps", bufs=1)
                            for dgt in range(4):
                                nc.tensor.matmul(
                                    ps[:], lhsT=outT[dgt][:, fc],
                                    rhs=wo[dgt][:, hf * LC:(hf + 1) * LC],
                                    start=(dgt == 0), stop=(dgt == 3))
                            nc.vector.tensor_copy(
                                yt[:, hf * LC:(hf + 1) * LC], ps[:])
                        nc.sync.dma_start(y_d[l0 + lt * 128:l0 + (lt + 1) * 128, :],
                                          yt[:])
    nc.compile()
    return nc


def _strip_dma_waits(nc):
    import concourse.mybir as mybir
    n2 = 0
    for f in nc.m.functions:
        for blk in f.blocks:
            for i in blk.instructions:
                if type(i).__name__ != 'InstDMACopy':
                    continue
                w = i.sync_info.on_wait
                if w and len(w) > 1:
                    keep = [x for x in w if not x.ant_name.startswith(('DMASW', 'DMAHW'))]
                    if not keep:
                        keep = [max(w, key=lambda x: x.wait_value)]
                    if len(keep) > 1:
                        n2 += 1
                    i.sync_info.on_wait = keep[:1] if len(keep) > 1 else keep
    if n2:
        print(f"_strip_dma_waits: {n2} DMAs had >1 compute wait (kept first)")


def _prep_inputs(inputs):
    x = np.asarray(inputs["x"], np.float32)
    E = np.asarray(inputs["E"], np.float32)
    F = np.asarray(inputs["F"], np.float32)
    ehi, elo = _split16(E)
    fhi, _ = _split16(F)
    efhi = np.concatenate([ehi, fhi], axis=1)
    se = E.sum(0).reshape(1, KP).astype(np.float16)
    sf = F.sum(0).reshape(1, KP).astype(np.float16)
    in_maps = []
    for c in range(8):
        b, g = c // 2, c % 2
        cols = slice(NG * g, NG * (g + 1))
        xhi, xlo = _split16(x[b])
        wqh, wql = _split16(np.asarray(inputs["Wq"], np.float32)[:, cols] * SCALE)
        wkh, wkl = _split16(np.asarray(inputs["Wk"], np.float32)[:, cols])
        wvh, _ = _split16(np.asarray(inputs["Wv"], np.float32)[:, cols])
        woh, _ = _split16(np.asarray(inputs["Wo"], np.float32)[cols, :])
        m = {
            "xhi": xhi, "xlo": xlo, "efhi": efhi, "elo": elo,
            "wqh": wqh, "wql": wql, "wkh": wkh, "wkl": wkl,
            "wv": wvh, "wo": woh,
            "bqs": (np.asarray(inputs["bq"], np.float32)[cols] * SCALE
                    ).reshape(1, NG).astype(np.float16),
            "bk": np.asarray(inputs["bk"], np.float32)[cols]
                    .reshape(1, NG).astype(np.float16),
            "bv": np.asarray(inputs["bv"], np.float32)[cols]
                    .reshape(1, NG).astype(np.float16),
            "se": se, "sf": sf,
        }
        in_maps.append({k: np.ascontiguousarray(v) for k, v in m.items()})
    return in_maps


def run(inputs, trace=False):
    from concourse.bass_utils import run_bass_kernel_spmd

    if "nc" not in _CACHE:
        _CACHE["nc"] = _build()
    nc = _CACHE["nc"]
    in_maps = _prep_inputs(inputs)
    res = run_bass_kernel_spmd(nc, in_maps, core_ids=list(range(8)), trace=trace)
    bo = np.asarray(inputs["bo"], np.float32)
    out = np.empty((B, L, D), np.float32)
    for b in range(B):
        out[b] = (res.results[2 * b]["y"].astype(np.float32)
                  + res.results[2 * b + 1]["y"].astype(np.float32) + bo)
    return out, res


def _host_reference(inputs):
    x = np.asarray(inputs["x"], np.float32)
    q = x @ inputs["Wq"] + inputs["bq"]
    k = x @ inputs["Wk"] + inputs["bk"]
    v = x @ inputs["Wv"] + inputs["bv"]
    Bs, Ls, Ds = x.shape
    q = q.reshape(Bs, Ls, H, DH); k = k.reshape(Bs, Ls, H, DH)
    v = v.reshape(Bs, Ls, H, DH)
    kE = np.einsum('blhd,lm->bhdm', k, np.asarray(inputs["E"], np.float32)[:Ls])
    vF = np.einsum('blhd,lm->bhmd', v, np.asarray(inputs["F"], np.float32)[:Ls])
    qk = np.einsum('blhd,bhdm->bhlm', q, kE) * SCALE
    qk -= qk.max(-1, keepdims=True)
    a = np.exp(qk); a /= a.sum(-1, keepdims=True)
    o = np.einsum('bhlm,bhmd->blhd', a, vF).reshape(Bs, Ls, Ds)
    return (o @ inputs["Wo"] + inputs["bo"]).astype(np.float32)


def kernel(**inputs):
    try:
        return run(inputs, trace=False)[0]
    except Exception:
        import traceback
        traceback.print_exc()
        return _host_reference(inputs)

